# revision 4
# baseline (speedup 1.0000x reference)
"""QRNN fo-pooling kernel for Trainium2 (Bass/Tile), batch-sharded across 8 cores.

Reference computation (per (b, h) element, sequential over t):
    F, Z, O = split(Y, 3, axis=2); F = sigmoid(F); Z = tanh(Z); O = sigmoid(O)
    c_t = F_t * c_{t-1} + (1 - F_t) * Z_t
    h_t = O_t * c_t
    out = concat([init_h, h], axis=0)

Mapping: the recurrence is a first-order linear scan -> DVE tensor_tensor_scan
(state = data0 * state + data1 along the free dim, fp32 state). Time must be on
the free dim, so raw F/Z are PE-transposed [t,h]->[h,t] (fp32 transpose mode),
activations run on ACT reading PSUM directly (doubling as the PSUM drain), the
scan runs per (b, h-block) over the full T=512, and c is PE-transposed back to
natural [t,h] layout where it is multiplied by sigmoid(O) and stored with
contiguous 512B rows.
"""

import numpy as np

import concourse.bacc as bacc
import concourse.bass as bass
import concourse.mybir as mybir
import concourse.tile as tile
from concourse.bass_utils import run_bass_kernel_spmd
from concourse.masks import make_identity

T, B, H = 512, 32, 1024
N_CORES = 8
BS = B // N_CORES  # batches per core
P = 128
HB = H // P  # h-blocks per core
TJ = T // P  # t-chunks

FP32 = mybir.dt.float32

_nc_cache = []


def _build_bass() -> bass.Bass:
    nc = bacc.Bacc("TRN2", target_bir_lowering=False)
    y = nc.declare_dram_parameter("Y", [T, BS, 3 * H], FP32, isOutput=False)
    init_c = nc.declare_dram_parameter("init_c", [1, BS, H], FP32, isOutput=False)
    init_h = nc.declare_dram_parameter("init_h", [1, BS, H], FP32, isOutput=False)
    out = nc.declare_dram_parameter("out", [T + 1, BS, H], FP32, isOutput=True)

    with tile.TileContext(nc) as tc:
        with (
            tc.tile_pool(name="sb", bufs=3) as sb,
            tc.tile_pool(name="psum", bufs=2, space="PSUM") as psum,
            tc.tile_pool(name="singles", bufs=1) as singles,
        ):
            ident = singles.tile([P, P], FP32)
            make_identity(nc, ident)

            # out[0] = init_h[0] (row 0 of the output is the initial h)
            nc.sync.dma_start(out=out[0, :, :], in_=init_h[0, :, :])

            # [t, b, c] -> [p, j, b, c] with t = j*128 + p
            yr = y[:, :, :].rearrange("(j p) b c -> p j b c", p=P)
            outr = out[1 : T + 1, :, :].rearrange("(j p) b h -> p j b h", p=P)
            ic_hb = init_c[0, :, :].rearrange("b h -> h b")

            for hb in range(HB):
                h0 = hb * P
                # per-partition initial state c_{-1}: [128 h, BS b]
                ic = sb.tile([P, BS], FP32, tag="ic")
                nc.sync.dma_start(out=ic, in_=ic_hb[h0 : h0 + P, :])

                for b in range(BS):
                    # natural-layout loads: [p=t%128, j=t//128, h] (512B rows)
                    f_raw = sb.tile([P, TJ, P], FP32, tag="f_raw")
                    z_raw = sb.tile([P, TJ, P], FP32, tag="z_raw")
                    o_raw = sb.tile([P, TJ, P], FP32, tag="o_raw")
                    nc.sync.dma_start(out=f_raw, in_=yr[:, :, b, h0 : h0 + P])
                    nc.sync.dma_start(out=z_raw, in_=yr[:, :, b, H + h0 : H + h0 + P])
                    nc.sync.dma_start(
                        out=o_raw, in_=yr[:, :, b, 2 * H + h0 : 2 * H + h0 + P]
                    )

                    # PE transpose raw F and Z: [t, h] -> [h, t], PSUM cols = t
                    ps_f = psum.tile([P, T], FP32, tag="ps_f")
                    ps_z = psum.tile([P, T], FP32, tag="ps_z")
                    for j in range(TJ):
                        nc.tensor.transpose(
                            ps_f[:, j * P : (j + 1) * P], f_raw[:, j, :], ident
                        )
                        nc.tensor.transpose(
                            ps_z[:, j * P : (j + 1) * P], z_raw[:, j, :], ident
                        )

                    # ACT reads PSUM, writes SBUF (doubles as PSUM drain):
                    # s_neg = sigmoid(-F_raw) = 1 - f ; zt = tanh(Z_raw)
                    s_neg = sb.tile([P, T], FP32, tag="s_neg")
                    nc.scalar.activation(
                        s_neg, ps_f[:, :], mybir.ActivationFunctionType.Sigmoid,
                        scale=-1.0,
                    )
                    zt = sb.tile([P, T], FP32, tag="zt")
                    nc.scalar.activation(
                        zt, ps_z[:, :], mybir.ActivationFunctionType.Tanh
                    )

                    # f = 1 - s_neg ; zf = (1 - f) * tanh(z) = s_neg * zt
                    f_t = sb.tile([P, T], FP32, tag="f_t")
                    nc.vector.tensor_scalar(
                        f_t, s_neg, -1.0, 1.0,
                        op0=mybir.AluOpType.mult, op1=mybir.AluOpType.add,
                    )
                    zf = sb.tile([P, T], FP32, tag="zf")
                    nc.vector.tensor_mul(zf, zt, s_neg)

                    # the recurrence: c[:, t] = f[:, t] * c[:, t-1] + zf[:, t]
                    c_t = sb.tile([P, T], FP32, tag="c_t")
                    nc.vector.tensor_tensor_scan(
                        c_t, f_t, zf, initial=ic[:, b : b + 1],
                        op0=mybir.AluOpType.mult, op1=mybir.AluOpType.add,
                    )

                    # transpose c back to natural layout: [h, t] -> [p=t%128, j, h]
                    ps_c = psum.tile([P, T], FP32, tag="ps_c")
                    for j in range(TJ):
                        nc.tensor.transpose(
                            ps_c[:, j * P : (j + 1) * P],
                            c_t[:, j * P : (j + 1) * P],
                            ident,
                        )

                    # h = sigmoid(O_raw) * c, all in natural layout
                    o_sig = sb.tile([P, TJ * P], FP32, tag="o_sig")
                    nc.scalar.activation(
                        o_sig, o_raw[:, :, :], mybir.ActivationFunctionType.Sigmoid
                    )
                    h_out = sb.tile([P, TJ * P], FP32, tag="h_out")
                    nc.vector.tensor_mul(h_out, o_sig, ps_c[:, :])

                    nc.sync.dma_start(
                        out=outr[:, :, b, h0 : h0 + P], in_=h_out
                    )
    nc.compile()
    return nc


def _get_nc() -> bass.Bass:
    if not _nc_cache:
        _nc_cache.append(_build_bass())
    return _nc_cache[0]


def kernel(Y: np.ndarray, init_c: np.ndarray, init_h: np.ndarray) -> np.ndarray:
    Y = np.ascontiguousarray(np.asarray(Y, dtype=np.float32))
    init_c = np.ascontiguousarray(np.asarray(init_c, dtype=np.float32))
    init_h = np.ascontiguousarray(np.asarray(init_h, dtype=np.float32))

    in_maps = []
    for k in range(N_CORES):
        sl = slice(k * BS, (k + 1) * BS)
        in_maps.append(
            {
                "Y": np.ascontiguousarray(Y[:, sl, :]),
                "init_c": np.ascontiguousarray(init_c[:, sl, :]),
                "init_h": np.ascontiguousarray(init_h[:, sl, :]),
            }
        )

    nc = _get_nc()
    res = run_bass_kernel_spmd(nc, in_maps, core_ids=list(range(N_CORES)))
    return np.concatenate([r["out"] for r in res.results], axis=1)
